# revision 2
# baseline (speedup 1.0000x reference)
"""Masked dot-product attention (B=64, Lq=Lk=1024, d=64, fp32) on 8 TRN2 cores.

Strategy (batch-parallel, 8 batch slots per core), v2:
  - Host folds the 1/sqrt(d) scale into Q and the additive key mask into an
    extra contraction row, so masked scores come out of one matmul:
        S^T[k, q] = sum_d K[k,d] * Q[q,d]/8 + maskadd[k]
    lhsT = ktm k-tile ([65, 128]), rhs = qt ([65, 1024]), fp32r.
  - Per k-tile, exp(S^T) is computed on ONE of two engines (greedy
    load-balanced):
      * ACT: exact exp, PSUM -> SBUF bf16.
      * DVE: Schraudolph-16 — int16(round(s*2^7*log2e + C)) written into a
        bf16 tile via bitcast; the int16 bit pattern IS the bf16 of
        ~0.971*2^(s*log2e) (max rel err ~3.3%).  fp32->int16 conversion
        saturates, so masked scores (-1e6) land at -32768 = bf16 -0.0,
        which contributes exactly nothing to numerator and denominator.
  - O accumulation per q-block j (8 blocks of 128 queries):
        opsum[q, 64j:64j+64] += P-block[k, q].T @ V-tile[k, :64]   (bf16)
        opsum[q, 512+j]      += P-block[k, q].T @ ones[k]          (denoms)
    64-wide bf16 matmuls cost half the columns of the fp32r orientation.
    PSUM keeps ONE open accumulation context per 2KB bank, so the kt loop
    must be INSIDE the j loop (groups sequential per bank); num lives in
    bank 6, den in bank 7 of the opsum tile.
  - Software pipelining: batch b's O-matmuls are emitted after batch b+1's
    first S/exp chunks so the exp engines stay fed during the O burst.
  - opsum [128, 520] is copied PSUM->SBUF (ACT or DVE, balanced) and DMA'd
    out unnormalized; host divides numerators by denominators.
  - Raggedness: k-tiles fully beyond valid_len are dead; batches are sorted
    by active-tile count and dealt across cores; per-slot tile counts baked
    into the program (compiled per distinct count tuple, cached).
  - valid_len==0 batches are host-patched (Q rows zeroed, mask zeroed) so
    scores are all 0 -> uniform attention, matching the reference.
"""

import numpy as np
import ml_dtypes

import concourse.bass as bass
import concourse.mybir as mybir
import concourse.tile as tile
from concourse import bacc
from concourse.bass_utils import run_bass_kernel_spmd

N_CORES = 8
B = 64
L = 1024
D = 64
BPC = B // N_CORES  # batch slots per core
KT = L // 128       # max k-tiles per batch
NEG_INF = -1000000.0

F32 = mybir.dt.float32
F32R = mybir.dt.float32r
BF16 = mybir.dt.bfloat16
I16 = mybir.dt.int16

# Schraudolph-16 constants: int16(round(s*SC1 + SC2)) bit-viewed as bf16
# approximates exp(s) within [0.968, 1.034]x.
SC1 = float(np.float32(184.66504))   # 2**7 * log2(e)
SC2 = float(np.float32(16250.51))    # 127*2**7 + 2**7*log2(0.9707) (minmax)

# Cost-model estimates (ns) used only for static ACT/DVE load balancing.
CHUNK = 512         # exp chunk = half a k-tile strip
ACT_CHUNK = CHUNK * 0.8333 + 185
DVE_CHUNK = CHUNK * 1.0417 + 125
ACT_COPY = 520 * 0.8333 + 185
DVE_COPY = 520 * 1.0417 + 125

_prog_cache = {}


def _build_program(ns):
    """ns: per-slot k-tile counts (tuple of BPC ints in 1..KT)."""
    nc = bacc.Bacc("TRN2", target_bir_lowering=False, debug=False,
                   num_devices=N_CORES)
    # qkt packs [ktm_ktile0 (128) | qt (1024) | ktm_ktile1.. (896)] so a
    # batch's whole Q/K working set arrives in one DMA.
    qkt_d = nc.dram_tensor("qkt", [BPC, D + 1, 2 * L + 128], F32R,
                           kind="ExternalInput")
    vp_d = nc.dram_tensor("vp", [BPC, 128, KT, D + 1], BF16,
                          kind="ExternalInput")
    o_d = nc.dram_tensor("o", [BPC, 128, 520], F32, kind="ExternalOutput")

    # Greedy two-engine balance.  ACT is pre-charged with its activation
    # table load (1.28us, overlaps input DMAs but delays its exp stream).
    busy = {"A": 1280.0, "V": 0.0}

    def pick(cost_a, cost_v):
        if busy["V"] + cost_v <= busy["A"] + cost_a:
            busy["V"] += cost_v
            return "V"
        busy["A"] += cost_a
        return "A"

    with tile.TileContext(nc) as tc:
        with (
            tc.tile_pool(name="qk", bufs=8) as qk_pool,
            tc.tile_pool(name="vpp", bufs=8) as vp_pool,
            tc.tile_pool(name="pt", bufs=28) as pt_pool,
            tc.tile_pool(name="osb", bufs=4) as osb_pool,
            tc.tile_pool(name="wu", bufs=1) as wu_pool,
            tc.tile_pool(name="sp", bufs=4, space="PSUM") as sp_pool,
            tc.tile_pool(name="op", bufs=2, space="PSUM") as op_pool,
        ):
            # PE p-state warmup: a dummy matmul as early as possible starts
            # the 3us ramp clock so real matmuls hit full speed sooner.
            wu = wu_pool.tile([1, 256], BF16)
            nc.vector.memset(wu[:], 0.0)
            wp = sp_pool.tile([128, CHUNK], F32, name="sp")
            nc.tensor.matmul(wp[:, :128], wu[:, :128], wu[:, 128:256],
                             start=True, stop=True)

            def make_o_closures(state):
                """Per-batch O-matmul emission, one j-group per closure.
                kt must be INSIDE j: PSUM keeps one open accumulation
                context per 2KB bank (num groups share bank A of opsum,
                den groups bank B)."""
                b, nkt, pts, vp_s, opsum, tail = state

                def o_group(j):
                    def emit():
                        w = 128 * (j % 4)
                        for kt in range(nkt):
                            first, last = kt == 0, kt == nkt - 1
                            pb = pts[2 * kt + j // 4][:, w:w + 128]
                            nc.tensor.matmul(
                                opsum[:, j * 64:(j + 1) * 64], pb,
                                vp_s[:, kt, :D], start=first, stop=last)
                            nc.tensor.matmul(
                                opsum[:, 512 + j:513 + j], pb,
                                vp_s[:, kt, D:D + 1], start=first, stop=last)
                    return emit

                def copy_out():
                    osb = osb_pool.tile([128, 520], F32)
                    if tail:
                        # tail batches: split the copy across both engines
                        nc.scalar.copy(osb[:, :256], opsum[:, :256])
                        nc.vector.tensor_copy(osb[:, 256:520],
                                              opsum[:, 256:520])
                    elif pick(ACT_COPY, DVE_COPY) == "A":
                        nc.scalar.copy(osb[:], opsum[:])
                    else:
                        nc.vector.tensor_copy(osb[:], opsum[:])
                    nc.sync.dma_start(o_d[b], osb[:])

                return [o_group(j) for j in range(8)] + [copy_out]

            gci = 0           # global chunk counter (startup engine forcing)
            prev = None       # completed batch awaiting O emission
            pending = []      # its closures, drained one per chunk
            for b in range(BPC):
                nkt = ns[b]
                tail = b >= BPC - 2
                end = 128 + L + (nkt - 1) * 128
                qkt_s = qk_pool.tile([D + 1, 2 * L + 128], F32R, tag="qkt")
                vp_s = vp_pool.tile([128, KT, D + 1], BF16)
                if b == 0:
                    # split head loads so the first chunks start asap
                    nc.sync.dma_start(qkt_s[:, :640], qkt_d[b][:, :640])
                    nc.sync.dma_start(qkt_s[:, 640:1152],
                                      qkt_d[b][:, 640:1152])
                    if end > 1152:
                        nc.sync.dma_start(qkt_s[:, 1152:end],
                                          qkt_d[b][:, 1152:end])
                else:
                    nc.sync.dma_start(qkt_s[:, :end], qkt_d[b][:, :end])
                nc.sync.dma_start(vp_s[:, :1, :], vp_d[b][:, :1, :])
                if nkt > 1:
                    nc.sync.dma_start(vp_s[:, 1:nkt, :], vp_d[b][:, 1:nkt, :])
                qt_s = qkt_s[:, 128:128 + L]

                def ktm_sl(kt):
                    if kt == 0:
                        return qkt_s[:, :128]
                    o = 128 + L + (kt - 1) * 128
                    return qkt_s[:, o:o + 128]

                opsum = op_pool.tile([128, 520], F32)
                pts = []
                for ci in range(2 * nkt):
                    kt, h = divmod(ci, 2)
                    sp = sp_pool.tile([128, CHUNK], F32)
                    pt = pt_pool.tile([128, CHUNK], BF16)
                    pts.append(pt)
                    nc.tensor.matmul(
                        sp[:], ktm_sl(kt), qt_s[:, h * 512:(h + 1) * 512],
                        start=True, stop=True)
                    # Slots with few active k-tiles hold small-valid_len
                    # batches: few softmax terms, so the Schraudolph ripple
                    # doesn't average out.  Keep those on exact ACT exp.
                    if nkt <= 2:
                        busy["A"] += ACT_CHUNK
                        eng = "A"
                    else:
                        eng = pick(ACT_CHUNK, DVE_CHUNK)
                    gci += 1
                    if eng == "A":
                        nc.scalar.activation(
                            pt[:], sp[:], mybir.ActivationFunctionType.Exp)
                    else:
                        nc.vector.tensor_scalar(
                            pt[:].bitcast(I16), sp[:], SC1, SC2,
                            mybir.AluOpType.mult, mybir.AluOpType.add)
                    if ci >= 1 and pending:
                        pending.pop(0)()
                while pending:
                    pending.pop(0)()
                prev = (b, nkt, pts, vp_s, opsum, tail)
                pending = make_o_closures(prev)

            # final batch: drain its O groups + copy-out
            for cl in pending:
                cl()

    nc.compile()
    return nc


def get_program(ns):
    ns = tuple(ns)
    if ns not in _prog_cache:
        _prog_cache[ns] = _build_program(ns)
    return _prog_cache[ns]


def _prep_inputs(q, k, v, vl):
    """q,k,v: [n, L, D] fp32; vl: [n] int. Returns (qkt, vp) arrays."""
    n = q.shape[0]
    qt = np.empty((n, D + 1, L), np.float32)
    qt[:, :D] = q.transpose(0, 2, 1) * np.float32(1.0 / np.sqrt(D))
    qt[:, D] = 1.0
    ktm = np.empty((n, D + 1, L), np.float32)
    ktm[:, :D] = k.transpose(0, 2, 1)
    iota = np.arange(L)
    ktm[:, D] = np.where(iota[None, :] < vl[:, None], 0.0, NEG_INF)
    # valid_len == 0: reference softmaxes a constant -1e6 row -> uniform.
    # Reproduce by zeroing the logits entirely (Q rows and mask row).
    zmask = vl == 0
    if zmask.any():
        qt[zmask, :D] = 0.0
        ktm[zmask, D] = 0.0
    qkt = np.empty((n, D + 1, 2 * L + 128), np.float32)
    qkt[:, :, :128] = ktm[:, :, :128]
    qkt[:, :, 128:128 + L] = qt
    qkt[:, :, 128 + L:2 * L] = ktm[:, :, 128:]
    qkt[:, :, 2 * L:] = 0.0
    vp = np.empty((n, L, D + 1), np.float32)
    vp[:, :, :D] = v
    vp[:, :, D] = 1.0
    vp = np.ascontiguousarray(
        vp.reshape(n, KT, 128, D + 1).transpose(0, 2, 1, 3))
    vp = vp.astype(ml_dtypes.bfloat16)  # [n, 128, KT, 65]
    return qkt, vp


def kernel(queries, keys, values, valid_lens):
    queries = np.asarray(queries, np.float32)
    keys = np.asarray(keys, np.float32)
    values = np.asarray(values, np.float32)
    vl = np.asarray(valid_lens).astype(np.int64)

    # Ragged load balancing: sort batches by active k-tile count descending,
    # deal them across cores (slot s <- sorted[s*N_CORES + c]), so each slot
    # runs the max tile count of its group of 8 on every core.
    nact = np.where(vl == 0, KT, -(-vl // 128)).astype(np.int64)
    order = np.argsort(-nact, kind="stable")
    ns = tuple(int(nact[order[s * N_CORES]]) for s in range(BPC))

    qkt, vp = _prep_inputs(queries[order], keys[order], values[order],
                           vl[order])

    nc = get_program(ns)
    in_maps = []
    for c in range(N_CORES):
        idx = [s * N_CORES + c for s in range(BPC)]
        in_maps.append({
            "qkt": np.ascontiguousarray(qkt[idx]),
            "vp": np.ascontiguousarray(vp[idx]),
        })

    res = None
    for attempt in range(3):
        try:
            res = run_bass_kernel_spmd(nc, in_maps, list(range(N_CORES)))
            break
        except Exception:
            # Transient NRT/axon device failures have been observed on the
            # first execution of a freshly compiled NEFF; reset and retry.
            if attempt == 2:
                raise
            import time as _time
            _time.sleep(2.0)
            try:
                import jax
                jax.clear_caches()
            except Exception:
                pass

    out = np.empty((B, L, D), np.float32)
    for c in range(N_CORES):
        o = res.results[c]["o"]  # [BPC, 128, 520]: 8x64 numerators + 8 denoms
        num = o[:, :, :512].reshape(BPC, 128, 8, D)
        den = o[:, :, 512:]
        on = (num / den[..., None]).transpose(0, 2, 1, 3).reshape(BPC, L, D)
        for s in range(BPC):
            out[order[s * N_CORES + c]] = on[s]
    return out


# revision 3
# speedup vs baseline: 1.0085x; 1.0085x over previous
"""Masked dot-product attention (B=64, Lq=Lk=1024, d=64, fp32) on 8 TRN2 cores.

Strategy (batch-parallel, 8 batch slots per core), v2:
  - Host folds the 1/sqrt(d) scale into Q and the additive key mask into an
    extra contraction row, so masked scores come out of one matmul:
        S^T[k, q] = sum_d K[k,d] * Q[q,d]/8 + maskadd[k]
    lhsT = ktm k-tile ([65, 128]), rhs = qt ([65, 1024]), fp32r.
  - Per k-tile, exp(S^T) is computed on ONE of two engines (greedy
    load-balanced):
      * ACT: exact exp, PSUM -> SBUF bf16.
      * DVE: Schraudolph-16 — int16(round(s*2^7*log2e + C)) written into a
        bf16 tile via bitcast; the int16 bit pattern IS the bf16 of
        ~0.971*2^(s*log2e) (max rel err ~3.3%).  fp32->int16 conversion
        saturates, so masked scores (-1e6) land at -32768 = bf16 -0.0,
        which contributes exactly nothing to numerator and denominator.
  - O accumulation per q-block j (8 blocks of 128 queries):
        opsum[q, 64j:64j+64] += P-block[k, q].T @ V-tile[k, :64]   (bf16)
        opsum[q, 512+j]      += P-block[k, q].T @ ones[k]          (denoms)
    64-wide bf16 matmuls cost half the columns of the fp32r orientation.
    PSUM keeps ONE open accumulation context per 2KB bank, so the kt loop
    must be INSIDE the j loop (groups sequential per bank); num lives in
    bank 6, den in bank 7 of the opsum tile.
  - Software pipelining: batch b's O-matmuls are emitted after batch b+1's
    first S/exp chunks so the exp engines stay fed during the O burst.
  - opsum [128, 520] is copied PSUM->SBUF (ACT or DVE, balanced) and DMA'd
    out unnormalized; host divides numerators by denominators.
  - Raggedness: k-tiles fully beyond valid_len are dead; batches are sorted
    by active-tile count and dealt across cores; per-slot tile counts baked
    into the program (compiled per distinct count tuple, cached).
  - valid_len==0 batches are host-patched (Q rows zeroed, mask zeroed) so
    scores are all 0 -> uniform attention, matching the reference.
"""

import numpy as np
import ml_dtypes

import concourse.bass as bass
import concourse.mybir as mybir
import concourse.tile as tile
from concourse import bacc
from concourse.bass_utils import run_bass_kernel_spmd

N_CORES = 8
B = 64
L = 1024
D = 64
BPC = B // N_CORES  # batch slots per core
KT = L // 128       # max k-tiles per batch
NEG_INF = -1000000.0

F32 = mybir.dt.float32
F32R = mybir.dt.float32r
BF16 = mybir.dt.bfloat16
I16 = mybir.dt.int16

# Schraudolph-16 constants: int16(round(s*SC1 + SC2)) bit-viewed as bf16
# approximates exp(s) within [0.968, 1.034]x.
SC1 = float(np.float32(184.66504))   # 2**7 * log2(e)
SC2 = float(np.float32(16250.51))    # 127*2**7 + 2**7*log2(0.9707) (minmax)

# Cost-model estimates (ns) used only for static ACT/DVE load balancing.
CHUNK = 512         # exp chunk = half a k-tile strip
ACT_CHUNK = CHUNK * 0.8333 + 185
DVE_CHUNK = CHUNK * 1.0417 + 125
ACT_COPY = 577 * 0.8333 + 185
DVE_COPY = 577 * 1.0417 + 125

_prog_cache = {}


def _build_program(ns):
    """ns: per-slot k-tile counts (tuple of BPC ints in 1..KT)."""
    nc = bacc.Bacc("TRN2", target_bir_lowering=False, debug=False,
                   num_devices=N_CORES)
    # qkt packs [ktm_ktile0 (128) | qt (1024) | ktm_ktile1.. (896)] so a
    # batch's whole Q/K working set arrives in one DMA.
    qkt_d = nc.dram_tensor("qkt", [BPC, D + 1, 2 * L + 128], F32R,
                           kind="ExternalInput")
    vp_d = nc.dram_tensor("vp", [BPC, 128, KT, D + 1], BF16,
                          kind="ExternalInput")
    o_d = nc.dram_tensor("o", [BPC, 128, 577], F32, kind="ExternalOutput")

    # Greedy two-engine balance.  ACT is pre-charged with its activation
    # table load (1.28us, overlaps input DMAs but delays its exp stream).
    busy = {"A": 1280.0, "V": 0.0}

    def pick(cost_a, cost_v):
        if busy["V"] + cost_v <= busy["A"] + cost_a:
            busy["V"] += cost_v
            return "V"
        busy["A"] += cost_a
        return "A"

    with tile.TileContext(nc) as tc:
        with (
            tc.tile_pool(name="qk", bufs=8) as qk_pool,
            tc.tile_pool(name="vpp", bufs=8) as vp_pool,
            tc.tile_pool(name="pt", bufs=28) as pt_pool,
            tc.tile_pool(name="osb", bufs=4) as osb_pool,
            tc.tile_pool(name="wu", bufs=1) as wu_pool,
            tc.tile_pool(name="sp", bufs=4, space="PSUM") as sp_pool,
            tc.tile_pool(name="op", bufs=2, space="PSUM") as op_pool,
        ):
            # PE p-state warmup: a dummy matmul as early as possible starts
            # the 3us ramp clock so real matmuls hit full speed sooner.
            wu = wu_pool.tile([1, 256], BF16)
            nc.vector.memset(wu[:], 0.0)
            wp = sp_pool.tile([128, CHUNK], F32, name="sp")
            nc.tensor.matmul(wp[:, :128], wu[:, :128], wu[:, 128:256],
                             start=True, stop=True)

            def make_o_closures(state):
                """Per-batch O-matmul emission, one j-group per closure.
                kt must be INSIDE j: PSUM keeps one open accumulation
                context per 2KB bank (num groups share bank A of opsum,
                den groups bank B)."""
                b, nkt, pts, vp_s, opsum, tail = state

                def o_group(j):
                    # 65-wide merged numerator+denominator matmuls.  Regions
                    # must not straddle a 2KB PSUM bank: j<7 pack into bank
                    # A at 65j (<= byte 1820), j=7 sits bank-aligned at 512.
                    off = 65 * j if j < 7 else 512

                    def emit():
                        w = 128 * (j % 4)
                        for kt in range(nkt):
                            first, last = kt == 0, kt == nkt - 1
                            pb = pts[2 * kt + j // 4][:, w:w + 128]
                            nc.tensor.matmul(
                                opsum[:, off:off + 65], pb,
                                vp_s[:, kt, :], start=first, stop=last)
                    return emit

                def copy_out():
                    osb = osb_pool.tile([128, 577], F32)
                    if tail:
                        # tail batches: split the copy across both engines
                        nc.scalar.copy(osb[:, :288], opsum[:, :288])
                        nc.vector.tensor_copy(osb[:, 288:577],
                                              opsum[:, 288:577])
                    elif pick(ACT_COPY, DVE_COPY) == "A":
                        nc.scalar.copy(osb[:], opsum[:])
                    else:
                        nc.vector.tensor_copy(osb[:], opsum[:])
                    nc.sync.dma_start(o_d[b], osb[:])

                return [o_group(j) for j in range(8)] + [copy_out]

            gci = 0           # global chunk counter (startup engine forcing)
            prev = None       # completed batch awaiting O emission
            pending = []      # its closures, drained one per chunk
            for b in range(BPC):
                nkt = ns[b]
                tail = b >= BPC - 2
                end = 128 + L + (nkt - 1) * 128
                qkt_s = qk_pool.tile([D + 1, 2 * L + 128], F32R, tag="qkt")
                vp_s = vp_pool.tile([128, KT, D + 1], BF16)
                if b == 0:
                    # split head loads so the first chunks start asap
                    nc.sync.dma_start(qkt_s[:, :640], qkt_d[b][:, :640])
                    nc.sync.dma_start(qkt_s[:, 640:1152],
                                      qkt_d[b][:, 640:1152])
                    if end > 1152:
                        nc.sync.dma_start(qkt_s[:, 1152:end],
                                          qkt_d[b][:, 1152:end])
                else:
                    nc.sync.dma_start(qkt_s[:, :end], qkt_d[b][:, :end])
                nc.sync.dma_start(vp_s[:, :1, :], vp_d[b][:, :1, :])
                if nkt > 1:
                    nc.sync.dma_start(vp_s[:, 1:nkt, :], vp_d[b][:, 1:nkt, :])
                qt_s = qkt_s[:, 128:128 + L]

                def ktm_sl(kt):
                    if kt == 0:
                        return qkt_s[:, :128]
                    o = 128 + L + (kt - 1) * 128
                    return qkt_s[:, o:o + 128]

                opsum = op_pool.tile([128, 577], F32)
                pts = []
                for ci in range(2 * nkt):
                    kt, h = divmod(ci, 2)
                    sp = sp_pool.tile([128, CHUNK], F32)
                    pt = pt_pool.tile([128, CHUNK], BF16)
                    pts.append(pt)
                    nc.tensor.matmul(
                        sp[:], ktm_sl(kt), qt_s[:, h * 512:(h + 1) * 512],
                        start=True, stop=True)
                    # Slots with few active k-tiles hold small-valid_len
                    # batches: few softmax terms, so the Schraudolph ripple
                    # doesn't average out.  Keep those on exact ACT exp.
                    if nkt <= 2:
                        busy["A"] += ACT_CHUNK
                        eng = "A"
                    else:
                        eng = pick(ACT_CHUNK, DVE_CHUNK)
                    gci += 1
                    if eng == "A":
                        nc.scalar.activation(
                            pt[:], sp[:], mybir.ActivationFunctionType.Exp)
                    else:
                        nc.vector.tensor_scalar(
                            pt[:].bitcast(I16), sp[:], SC1, SC2,
                            mybir.AluOpType.mult, mybir.AluOpType.add)
                    if ci >= 3 and pending:
                        pending.pop(0)()
                while pending:
                    pending.pop(0)()
                prev = (b, nkt, pts, vp_s, opsum, tail)
                pending = make_o_closures(prev)

            # final batch: drain its O groups + copy-out
            for cl in pending:
                cl()

    nc.compile()
    return nc


def get_program(ns):
    ns = tuple(ns)
    if ns not in _prog_cache:
        _prog_cache[ns] = _build_program(ns)
    return _prog_cache[ns]


def _prep_inputs(q, k, v, vl):
    """q,k,v: [n, L, D] fp32; vl: [n] int. Returns (qkt, vp) arrays."""
    n = q.shape[0]
    qt = np.empty((n, D + 1, L), np.float32)
    qt[:, :D] = q.transpose(0, 2, 1) * np.float32(1.0 / np.sqrt(D))
    qt[:, D] = 1.0
    ktm = np.empty((n, D + 1, L), np.float32)
    ktm[:, :D] = k.transpose(0, 2, 1)
    iota = np.arange(L)
    ktm[:, D] = np.where(iota[None, :] < vl[:, None], 0.0, NEG_INF)
    # valid_len == 0: reference softmaxes a constant -1e6 row -> uniform.
    # Reproduce by zeroing the logits entirely (Q rows and mask row).
    zmask = vl == 0
    if zmask.any():
        qt[zmask, :D] = 0.0
        ktm[zmask, D] = 0.0
    qkt = np.empty((n, D + 1, 2 * L + 128), np.float32)
    qkt[:, :, :128] = ktm[:, :, :128]
    qkt[:, :, 128:128 + L] = qt
    qkt[:, :, 128 + L:2 * L] = ktm[:, :, 128:]
    qkt[:, :, 2 * L:] = 0.0
    vp = np.empty((n, L, D + 1), np.float32)
    vp[:, :, :D] = v
    vp[:, :, D] = 1.0
    vp = np.ascontiguousarray(
        vp.reshape(n, KT, 128, D + 1).transpose(0, 2, 1, 3))
    vp = vp.astype(ml_dtypes.bfloat16)  # [n, 128, KT, 65]
    return qkt, vp


def kernel(queries, keys, values, valid_lens):
    queries = np.asarray(queries, np.float32)
    keys = np.asarray(keys, np.float32)
    values = np.asarray(values, np.float32)
    vl = np.asarray(valid_lens).astype(np.int64)

    # Ragged load balancing: sort batches by active k-tile count descending,
    # deal them across cores (slot s <- sorted[s*N_CORES + c]), so each slot
    # runs the max tile count of its group of 8 on every core.
    nact = np.where(vl == 0, KT, -(-vl // 128)).astype(np.int64)
    order = np.argsort(-nact, kind="stable")
    ns = tuple(int(nact[order[s * N_CORES]]) for s in range(BPC))

    qkt, vp = _prep_inputs(queries[order], keys[order], values[order],
                           vl[order])

    nc = get_program(ns)
    in_maps = []
    for c in range(N_CORES):
        idx = [s * N_CORES + c for s in range(BPC)]
        in_maps.append({
            "qkt": np.ascontiguousarray(qkt[idx]),
            "vp": np.ascontiguousarray(vp[idx]),
        })

    res = None
    for attempt in range(3):
        try:
            res = run_bass_kernel_spmd(nc, in_maps, list(range(N_CORES)))
            break
        except Exception:
            # Transient NRT/axon device failures have been observed on the
            # first execution of a freshly compiled NEFF; reset and retry.
            if attempt == 2:
                raise
            import time as _time
            _time.sleep(2.0)
            try:
                import jax
                jax.clear_caches()
            except Exception:
                pass

    out = np.empty((B, L, D), np.float32)
    for c in range(N_CORES):
        o = res.results[c]["o"]  # [BPC, 128, 577]: 8x(64 num + 1 den) regions
        lo = o[:, :, :455].reshape(BPC, 128, 7, 65)
        num = np.concatenate([lo[..., :64], o[:, :, None, 512:576]], axis=2)
        den = np.concatenate([lo[..., 64], o[:, :, None, 576]], axis=2)
        on = (num / den[..., None]).transpose(0, 2, 1, 3).reshape(BPC, L, D)
        for s in range(BPC):
            out[order[s * N_CORES + c]] = on[s]
    return out


# revision 5
# speedup vs baseline: 1.0236x; 1.0150x over previous
"""Masked dot-product attention (B=64, Lq=Lk=1024, d=64, fp32) on 8 TRN2 cores.

Strategy (batch-parallel, 8 batch slots per core), v2:
  - Host folds the 1/sqrt(d) scale into Q and the additive key mask into an
    extra contraction row, so masked scores come out of one matmul:
        S^T[k, q] = sum_d K[k,d] * Q[q,d]/8 + maskadd[k]
    lhsT = ktm k-tile ([65, 128]), rhs = qt ([65, 1024]), fp32r.
  - Per k-tile, exp(S^T) is computed on ONE of two engines (greedy
    load-balanced):
      * ACT: exact exp, PSUM -> SBUF bf16.
      * DVE: Schraudolph-16 — int16(round(s*2^7*log2e + C)) written into a
        bf16 tile via bitcast; the int16 bit pattern IS the bf16 of
        ~0.971*2^(s*log2e) (max rel err ~3.3%).  fp32->int16 conversion
        saturates, so masked scores (-1e6) land at -32768 = bf16 -0.0,
        which contributes exactly nothing to numerator and denominator.
  - O accumulation per q-block j (8 blocks of 128 queries):
        opsum[q, 64j:64j+64] += P-block[k, q].T @ V-tile[k, :64]   (bf16)
        opsum[q, 512+j]      += P-block[k, q].T @ ones[k]          (denoms)
    64-wide bf16 matmuls cost half the columns of the fp32r orientation.
    PSUM keeps ONE open accumulation context per 2KB bank, so the kt loop
    must be INSIDE the j loop (groups sequential per bank); num lives in
    bank 6, den in bank 7 of the opsum tile.
  - Software pipelining: batch b's O-matmuls are emitted after batch b+1's
    first S/exp chunks so the exp engines stay fed during the O burst.
  - opsum [128, 520] is copied PSUM->SBUF (ACT or DVE, balanced) and DMA'd
    out unnormalized; host divides numerators by denominators.
  - Raggedness: k-tiles fully beyond valid_len are dead; batches are sorted
    by active-tile count and dealt across cores; per-slot tile counts baked
    into the program (compiled per distinct count tuple, cached).
  - valid_len==0 batches are host-patched (Q rows zeroed, mask zeroed) so
    scores are all 0 -> uniform attention, matching the reference.
"""

import numpy as np
import ml_dtypes

import concourse.bass as bass
import concourse.mybir as mybir
import concourse.tile as tile
from concourse import bacc
from concourse.bass_utils import run_bass_kernel_spmd

N_CORES = 8
B = 64
L = 1024
D = 64
BPC = B // N_CORES  # batch slots per core
KT = L // 128       # max k-tiles per batch
NEG_INF = -1000000.0

F32 = mybir.dt.float32
F32R = mybir.dt.float32r
BF16 = mybir.dt.bfloat16
I16 = mybir.dt.int16

# Schraudolph-16 constants: int16(round(s*SC1 + SC2)) bit-viewed as bf16
# approximates exp(s) within [0.968, 1.034]x.
SC1 = float(np.float32(184.66504))   # 2**7 * log2(e)
SC2 = float(np.float32(16250.51))    # 127*2**7 + 2**7*log2(0.9707) (minmax)

# Cost-model estimates (ns) used only for static ACT/DVE load balancing.
CHUNK = 512         # exp chunk = half a k-tile strip
ACT_CHUNK = CHUNK * 0.8333 + 185
DVE_CHUNK = CHUNK * 1.0417 + 125
ACT_COPY = 577 * 0.8333 + 185
DVE_COPY = 577 * 1.0417 + 125

_prog_cache = {}


def _build_program(ns):
    """ns: per-slot k-tile counts (tuple of BPC ints in 1..KT)."""
    nc = bacc.Bacc("TRN2", target_bir_lowering=False, debug=False,
                   num_devices=N_CORES)
    # qkt packs [ktm_ktile0 (128) | qt (1024) | ktm_ktile1.. (896)] so a
    # batch's whole Q/K working set arrives in one DMA.
    qkt_d = nc.dram_tensor("qkt", [BPC, D + 1, 2 * L + 128], F32R,
                           kind="ExternalInput")
    vp_d = nc.dram_tensor("vp", [BPC, 128, KT, D + 1], BF16,
                          kind="ExternalInput")
    o_d = nc.dram_tensor("o", [BPC, 128, 577], F32, kind="ExternalOutput")

    # Greedy two-engine balance.  ACT is pre-charged with its activation
    # table load (1.28us, overlaps input DMAs but delays its exp stream).
    busy = {"A": 1280.0, "V": 0.0}

    def pick(cost_a, cost_v):
        if busy["V"] + cost_v <= busy["A"] + cost_a:
            busy["V"] += cost_v
            return "V"
        busy["A"] += cost_a
        return "A"

    with tile.TileContext(nc) as tc:
        with (
            tc.tile_pool(name="qk", bufs=8) as qk_pool,
            tc.tile_pool(name="vpp", bufs=8) as vp_pool,
            tc.tile_pool(name="pt", bufs=28) as pt_pool,
            tc.tile_pool(name="osb", bufs=4) as osb_pool,
            tc.tile_pool(name="wu", bufs=1) as wu_pool,
            tc.tile_pool(name="sp", bufs=4, space="PSUM") as sp_pool,
            tc.tile_pool(name="op", bufs=2, space="PSUM") as op_pool,
        ):
            # PE p-state warmup: a dummy matmul as early as possible starts
            # the 3us ramp clock so real matmuls hit full speed sooner.
            wu = wu_pool.tile([1, 256], BF16)
            nc.vector.memset(wu[:], 0.0)
            wp = sp_pool.tile([128, CHUNK], F32, name="sp")
            nc.tensor.matmul(wp[:, :128], wu[:, :128], wu[:, 128:256],
                             start=True, stop=True)

            def make_o_closures(state):
                """Per-batch O-matmul emission, one j-group per closure.
                kt must be INSIDE j: PSUM keeps one open accumulation
                context per 2KB bank (num groups share bank A of opsum,
                den groups bank B)."""
                b, nkt, pts, vp_s, opsum, tail = state

                def o_group(j):
                    # 65-wide merged numerator+denominator matmuls.  Regions
                    # must not straddle a 2KB PSUM bank: j<7 pack into bank
                    # A at 65j (<= byte 1820), j=7 sits bank-aligned at 512.
                    off = 65 * j if j < 7 else 512

                    def emit():
                        w = 128 * (j % 4)
                        for kt in range(nkt):
                            first, last = kt == 0, kt == nkt - 1
                            pb = pts[2 * kt + j // 4][:, w:w + 128]
                            nc.tensor.matmul(
                                opsum[:, off:off + 65], pb,
                                vp_s[:, kt, :], start=first, stop=last)
                    return emit

                def copy_out():
                    osb = osb_pool.tile([128, 577], F32)
                    if tail:
                        # tail batches: whole copy on one engine each (no
                        # cross-engine rendezvous before the out-DMA); ACT
                        # finishes its exps first and takes the earlier one
                        if b == BPC - 2:
                            nc.scalar.copy(osb[:], opsum[:])
                        else:
                            nc.vector.tensor_copy(osb[:], opsum[:])
                    else:
                        # DVE's mid-stream idle absorbs copies for free;
                        # keep ACT's (critical) exp stream copy-free
                        nc.vector.tensor_copy(osb[:], opsum[:])
                    nc.sync.dma_start(o_d[b], osb[:])

                return [o_group(j) for j in range(8)] + [copy_out]

            gci = 0           # global chunk counter (startup engine forcing)
            prev = None       # completed batch awaiting O emission
            pending = []      # its closures, drained one per chunk
            for b in range(BPC):
                nkt = ns[b]
                tail = b >= BPC - 2
                end = 128 + L + (nkt - 1) * 128
                qkt_s = qk_pool.tile([D + 1, 2 * L + 128], F32R, tag="qkt")
                vp_s = vp_pool.tile([128, KT, D + 1], BF16)
                if b == 0:
                    # split head loads so the first chunks start asap
                    nc.sync.dma_start(qkt_s[:, :640], qkt_d[b][:, :640])
                    nc.sync.dma_start(qkt_s[:, 640:1152],
                                      qkt_d[b][:, 640:1152])
                    if end > 1152:
                        nc.sync.dma_start(qkt_s[:, 1152:end],
                                          qkt_d[b][:, 1152:end])
                else:
                    nc.sync.dma_start(qkt_s[:, :end], qkt_d[b][:, :end])
                nc.sync.dma_start(vp_s[:, :1, :], vp_d[b][:, :1, :])
                if nkt > 1:
                    nc.sync.dma_start(vp_s[:, 1:nkt, :], vp_d[b][:, 1:nkt, :])
                qt_s = qkt_s[:, 128:128 + L]

                def ktm_sl(kt):
                    if kt == 0:
                        return qkt_s[:, :128]
                    o = 128 + L + (kt - 1) * 128
                    return qkt_s[:, o:o + 128]

                opsum = op_pool.tile([128, 577], F32)
                pts = []
                for ci in range(2 * nkt):
                    kt, h = divmod(ci, 2)
                    sp = sp_pool.tile([128, CHUNK], F32)
                    pt = pt_pool.tile([128, CHUNK], BF16)
                    pts.append(pt)
                    nc.tensor.matmul(
                        sp[:], ktm_sl(kt), qt_s[:, h * 512:(h + 1) * 512],
                        start=True, stop=True)
                    # Slots with few active k-tiles hold small-valid_len
                    # batches: few softmax terms, so the Schraudolph ripple
                    # doesn't average out.  Keep those on exact ACT exp.
                    if nkt <= 2:
                        busy["A"] += ACT_CHUNK
                        eng = "A"
                    else:
                        eng = pick(ACT_CHUNK, DVE_CHUNK)
                    gci += 1
                    if eng == "A":
                        nc.scalar.activation(
                            pt[:], sp[:], mybir.ActivationFunctionType.Exp)
                    else:
                        nc.vector.tensor_scalar(
                            pt[:].bitcast(I16), sp[:], SC1, SC2,
                            mybir.AluOpType.mult, mybir.AluOpType.add)
                    if ci >= 3 and pending:
                        pending.pop(0)()
                while pending:
                    pending.pop(0)()
                prev = (b, nkt, pts, vp_s, opsum, tail)
                pending = make_o_closures(prev)

            # final batch: drain its O groups + copy-out
            for cl in pending:
                cl()

    nc.compile()
    return nc


def get_program(ns):
    ns = tuple(ns)
    if ns not in _prog_cache:
        _prog_cache[ns] = _build_program(ns)
    return _prog_cache[ns]


def _prep_inputs(q, k, v, vl):
    """q,k,v: [n, L, D] fp32; vl: [n] int. Returns (qkt, vp) arrays."""
    n = q.shape[0]
    qt = np.empty((n, D + 1, L), np.float32)
    qt[:, :D] = q.transpose(0, 2, 1) * np.float32(1.0 / np.sqrt(D))
    qt[:, D] = 1.0
    ktm = np.empty((n, D + 1, L), np.float32)
    ktm[:, :D] = k.transpose(0, 2, 1)
    iota = np.arange(L)
    ktm[:, D] = np.where(iota[None, :] < vl[:, None], 0.0, NEG_INF)
    # valid_len == 0: reference softmaxes a constant -1e6 row -> uniform.
    # Reproduce by zeroing the logits entirely (Q rows and mask row).
    zmask = vl == 0
    if zmask.any():
        qt[zmask, :D] = 0.0
        ktm[zmask, D] = 0.0
    qkt = np.empty((n, D + 1, 2 * L + 128), np.float32)
    qkt[:, :, :128] = ktm[:, :, :128]
    qkt[:, :, 128:128 + L] = qt
    qkt[:, :, 128 + L:2 * L] = ktm[:, :, 128:]
    qkt[:, :, 2 * L:] = 0.0
    vp = np.empty((n, L, D + 1), np.float32)
    vp[:, :, :D] = v
    vp[:, :, D] = 1.0
    vp = np.ascontiguousarray(
        vp.reshape(n, KT, 128, D + 1).transpose(0, 2, 1, 3))
    vp = vp.astype(ml_dtypes.bfloat16)  # [n, 128, KT, 65]
    return qkt, vp


def kernel(queries, keys, values, valid_lens):
    queries = np.asarray(queries, np.float32)
    keys = np.asarray(keys, np.float32)
    values = np.asarray(values, np.float32)
    vl = np.asarray(valid_lens).astype(np.int64)

    # Ragged load balancing: sort batches by active k-tile count descending,
    # deal them across cores (slot s <- sorted[s*N_CORES + c]), so each slot
    # runs the max tile count of its group of 8 on every core.
    nact = np.where(vl == 0, KT, -(-vl // 128)).astype(np.int64)
    order = np.argsort(-nact, kind="stable")
    ns = tuple(int(nact[order[s * N_CORES]]) for s in range(BPC))

    qkt, vp = _prep_inputs(queries[order], keys[order], values[order],
                           vl[order])

    nc = get_program(ns)
    in_maps = []
    for c in range(N_CORES):
        idx = [s * N_CORES + c for s in range(BPC)]
        in_maps.append({
            "qkt": np.ascontiguousarray(qkt[idx]),
            "vp": np.ascontiguousarray(vp[idx]),
        })

    res = None
    for attempt in range(3):
        try:
            res = run_bass_kernel_spmd(nc, in_maps, list(range(N_CORES)))
            break
        except Exception:
            # Transient NRT/axon device failures have been observed on the
            # first execution of a freshly compiled NEFF; reset and retry.
            if attempt == 2:
                raise
            import time as _time
            _time.sleep(2.0)
            try:
                import jax
                jax.clear_caches()
            except Exception:
                pass

    out = np.empty((B, L, D), np.float32)
    for c in range(N_CORES):
        o = res.results[c]["o"]  # [BPC, 128, 577]: 8x(64 num + 1 den) regions
        lo = o[:, :, :455].reshape(BPC, 128, 7, 65)
        num = np.concatenate([lo[..., :64], o[:, :, None, 512:576]], axis=2)
        den = np.concatenate([lo[..., 64], o[:, :, None, 576]], axis=2)
        on = (num / den[..., None]).transpose(0, 2, 1, 3).reshape(BPC, L, D)
        for s in range(BPC):
            out[order[s * N_CORES + c]] = on[s]
    return out


# revision 6
# speedup vs baseline: 1.0314x; 1.0076x over previous
"""Masked dot-product attention (B=64, Lq=Lk=1024, d=64, fp32) on 8 TRN2 cores.

Strategy (batch-parallel, 8 batch slots per core), v2:
  - Host folds the 1/sqrt(d) scale into Q and the additive key mask into an
    extra contraction row, so masked scores come out of one matmul:
        S^T[k, q] = sum_d K[k,d] * Q[q,d]/8 + maskadd[k]
    lhsT = ktm k-tile ([65, 128]), rhs = qt ([65, 1024]), fp32r.
  - Per k-tile, exp(S^T) is computed on ONE of two engines (greedy
    load-balanced):
      * ACT: exact exp, PSUM -> SBUF bf16.
      * DVE: Schraudolph-16 — int16(round(s*2^7*log2e + C)) written into a
        bf16 tile via bitcast; the int16 bit pattern IS the bf16 of
        ~0.971*2^(s*log2e) (max rel err ~3.3%).  fp32->int16 conversion
        saturates, so masked scores (-1e6) land at -32768 = bf16 -0.0,
        which contributes exactly nothing to numerator and denominator.
  - O accumulation per q-block j (8 blocks of 128 queries):
        opsum[q, 64j:64j+64] += P-block[k, q].T @ V-tile[k, :64]   (bf16)
        opsum[q, 512+j]      += P-block[k, q].T @ ones[k]          (denoms)
    64-wide bf16 matmuls cost half the columns of the fp32r orientation.
    PSUM keeps ONE open accumulation context per 2KB bank, so the kt loop
    must be INSIDE the j loop (groups sequential per bank); num lives in
    bank 6, den in bank 7 of the opsum tile.
  - Software pipelining: batch b's O-matmuls are emitted after batch b+1's
    first S/exp chunks so the exp engines stay fed during the O burst.
  - opsum [128, 520] is copied PSUM->SBUF (ACT or DVE, balanced) and DMA'd
    out unnormalized; host divides numerators by denominators.
  - Raggedness: k-tiles fully beyond valid_len are dead; batches are sorted
    by active-tile count and dealt across cores; per-slot tile counts baked
    into the program (compiled per distinct count tuple, cached).
  - valid_len==0 batches are host-patched (Q rows zeroed, mask zeroed) so
    scores are all 0 -> uniform attention, matching the reference.
"""

import numpy as np
import ml_dtypes

import concourse.bass as bass
import concourse.mybir as mybir
import concourse.tile as tile
from concourse import bacc
from concourse.bass_utils import run_bass_kernel_spmd

N_CORES = 8
B = 64
L = 1024
D = 64
BPC = B // N_CORES  # batch slots per core
KT = L // 128       # max k-tiles per batch
NEG_INF = -1000000.0

F32 = mybir.dt.float32
F32R = mybir.dt.float32r
BF16 = mybir.dt.bfloat16
I16 = mybir.dt.int16

# Schraudolph-16 constants: int16(round(s*SC1 + SC2)) bit-viewed as bf16
# approximates exp(s) within [0.968, 1.034]x.
SC1 = float(np.float32(184.66504))   # 2**7 * log2(e)
SC2 = float(np.float32(16250.51))    # 127*2**7 + 2**7*log2(0.9707) (minmax)

# Cost-model estimates (ns) used only for static ACT/DVE load balancing.
CHUNK = 512         # exp chunk = half a k-tile strip
ACT_CHUNK = CHUNK * 0.8333 + 185
DVE_CHUNK = CHUNK * 1.0417 + 125
ACT_COPY = 577 * 0.8333 + 185
DVE_COPY = 577 * 1.0417 + 125

_prog_cache = {}


def _build_program(ns):
    """ns: per-slot k-tile counts (tuple of BPC ints in 1..KT)."""
    nc = bacc.Bacc("TRN2", target_bir_lowering=False, debug=False,
                   num_devices=N_CORES)
    # qkt packs [ktm_ktile0 (128) | qt (1024) | ktm_ktile1.. (896)] so a
    # batch's whole Q/K working set arrives in one DMA.
    qkt_d = nc.dram_tensor("qkt", [BPC, D + 1, 2 * L + 128], F32R,
                           kind="ExternalInput")
    vp_d = nc.dram_tensor("vp", [BPC, 128, KT, D + 1], BF16,
                          kind="ExternalInput")
    o_d = nc.dram_tensor("o", [BPC, 128, 577], F32, kind="ExternalOutput")

    # Greedy two-engine balance.  ACT is pre-charged with its activation
    # table load (1.28us, overlaps input DMAs but delays its exp stream).
    busy = {"A": 1280.0, "V": 0.0}

    def pick(cost_a, cost_v):
        if busy["V"] + cost_v <= busy["A"] + cost_a:
            busy["V"] += cost_v
            return "V"
        busy["A"] += cost_a
        return "A"

    with tile.TileContext(nc) as tc:
        with (
            tc.tile_pool(name="qk", bufs=8) as qk_pool,
            tc.tile_pool(name="vpp", bufs=8) as vp_pool,
            tc.tile_pool(name="pt", bufs=28) as pt_pool,
            tc.tile_pool(name="osb", bufs=4) as osb_pool,
            tc.tile_pool(name="wu", bufs=1) as wu_pool,
            tc.tile_pool(name="sp", bufs=4, space="PSUM") as sp_pool,
            tc.tile_pool(name="op", bufs=2, space="PSUM") as op_pool,
        ):
            # PE p-state warmup: a dummy matmul as early as possible starts
            # the 3us ramp clock so real matmuls hit full speed sooner.
            wu = wu_pool.tile([1, 256], BF16)
            nc.vector.memset(wu[:], 0.0)
            wp = sp_pool.tile([128, CHUNK], F32, name="sp")
            nc.tensor.matmul(wp[:, :128], wu[:, :128], wu[:, 128:256],
                             start=True, stop=True)

            def make_o_closures(state):
                """Per-batch O-matmul emission, one j-group per closure.
                kt must be INSIDE j: PSUM keeps one open accumulation
                context per 2KB bank (num groups share bank A of opsum,
                den groups bank B)."""
                b, nkt, pts, vp_s, opsum, tail = state

                def o_group(j):
                    # 65-wide merged numerator+denominator matmuls.  Regions
                    # must not straddle a 2KB PSUM bank: j<7 pack into bank
                    # A at 65j (<= byte 1820), j=7 sits bank-aligned at 512.
                    off = 65 * j if j < 7 else 512

                    def emit():
                        w = 128 * (j % 4)
                        for kt in range(nkt):
                            first, last = kt == 0, kt == nkt - 1
                            pb = pts[2 * kt + j // 4][:, w:w + 128]
                            nc.tensor.matmul(
                                opsum[:, off:off + 65], pb,
                                vp_s[:, kt, :], start=first, stop=last)
                    return emit

                def copy_out():
                    osb = osb_pool.tile([128, 577], F32)
                    if tail:
                        # tail batches: whole copy on one engine each (no
                        # cross-engine rendezvous before the out-DMA); ACT
                        # finishes its exps first and takes the earlier one
                        if b == BPC - 2:
                            nc.vector.tensor_copy(osb[:], opsum[:])
                        else:
                            nc.scalar.copy(osb[:], opsum[:])
                    else:
                        # DVE's mid-stream idle absorbs copies for free;
                        # keep ACT's (critical) exp stream copy-free
                        nc.vector.tensor_copy(osb[:], opsum[:])
                    nc.sync.dma_start(o_d[b], osb[:])

                return [o_group(j) for j in range(8)] + [copy_out]

            gci = 0           # global chunk counter (startup engine forcing)
            prev = None       # completed batch awaiting O emission
            pending = []      # its closures, drained one per chunk
            for b in range(BPC):
                nkt = ns[b]
                tail = b >= BPC - 2
                end = 128 + L + (nkt - 1) * 128
                qkt_s = qk_pool.tile([D + 1, 2 * L + 128], F32R, tag="qkt")
                vp_s = vp_pool.tile([128, KT, D + 1], BF16)
                if b == 0:
                    # split head loads so the first chunks start asap
                    nc.sync.dma_start(qkt_s[:, :640], qkt_d[b][:, :640])
                    nc.sync.dma_start(qkt_s[:, 640:1152],
                                      qkt_d[b][:, 640:1152])
                    if end > 1152:
                        nc.sync.dma_start(qkt_s[:, 1152:end],
                                          qkt_d[b][:, 1152:end])
                else:
                    nc.sync.dma_start(qkt_s[:, :end], qkt_d[b][:, :end])
                nc.sync.dma_start(vp_s[:, :1, :], vp_d[b][:, :1, :])
                if nkt > 1:
                    nc.sync.dma_start(vp_s[:, 1:nkt, :], vp_d[b][:, 1:nkt, :])
                qt_s = qkt_s[:, 128:128 + L]

                def ktm_sl(kt):
                    if kt == 0:
                        return qkt_s[:, :128]
                    o = 128 + L + (kt - 1) * 128
                    return qkt_s[:, o:o + 128]

                opsum = op_pool.tile([128, 577], F32)
                pts = []
                for ci in range(2 * nkt):
                    kt, h = divmod(ci, 2)
                    sp = sp_pool.tile([128, CHUNK], F32)
                    pt = pt_pool.tile([128, CHUNK], BF16)
                    pts.append(pt)
                    nc.tensor.matmul(
                        sp[:], ktm_sl(kt), qt_s[:, h * 512:(h + 1) * 512],
                        start=True, stop=True)
                    # Slots with few active k-tiles hold small-valid_len
                    # batches: few softmax terms, so the Schraudolph ripple
                    # doesn't average out.  Keep those on exact ACT exp.
                    if nkt <= 2:
                        busy["A"] += ACT_CHUNK
                        eng = "A"
                    else:
                        eng = pick(ACT_CHUNK, DVE_CHUNK)
                    gci += 1
                    if eng == "A":
                        nc.scalar.activation(
                            pt[:], sp[:], mybir.ActivationFunctionType.Exp)
                    else:
                        nc.vector.tensor_scalar(
                            pt[:].bitcast(I16), sp[:], SC1, SC2,
                            mybir.AluOpType.mult, mybir.AluOpType.add)
                    if ci >= 3 and pending:
                        pending.pop(0)()
                while pending:
                    pending.pop(0)()
                prev = (b, nkt, pts, vp_s, opsum, tail)
                pending = make_o_closures(prev)

            # final batch: drain its O groups + copy-out
            for cl in pending:
                cl()

    nc.compile()
    return nc


def get_program(ns):
    ns = tuple(ns)
    if ns not in _prog_cache:
        _prog_cache[ns] = _build_program(ns)
    return _prog_cache[ns]


def _prep_inputs(q, k, v, vl):
    """q,k,v: [n, L, D] fp32; vl: [n] int. Returns (qkt, vp) arrays."""
    n = q.shape[0]
    qt = np.empty((n, D + 1, L), np.float32)
    qt[:, :D] = q.transpose(0, 2, 1) * np.float32(1.0 / np.sqrt(D))
    qt[:, D] = 1.0
    ktm = np.empty((n, D + 1, L), np.float32)
    ktm[:, :D] = k.transpose(0, 2, 1)
    iota = np.arange(L)
    ktm[:, D] = np.where(iota[None, :] < vl[:, None], 0.0, NEG_INF)
    # valid_len == 0: reference softmaxes a constant -1e6 row -> uniform.
    # Reproduce by zeroing the logits entirely (Q rows and mask row).
    zmask = vl == 0
    if zmask.any():
        qt[zmask, :D] = 0.0
        ktm[zmask, D] = 0.0
    qkt = np.empty((n, D + 1, 2 * L + 128), np.float32)
    qkt[:, :, :128] = ktm[:, :, :128]
    qkt[:, :, 128:128 + L] = qt
    qkt[:, :, 128 + L:2 * L] = ktm[:, :, 128:]
    qkt[:, :, 2 * L:] = 0.0
    vp = np.empty((n, L, D + 1), np.float32)
    vp[:, :, :D] = v
    vp[:, :, D] = 1.0
    vp = np.ascontiguousarray(
        vp.reshape(n, KT, 128, D + 1).transpose(0, 2, 1, 3))
    vp = vp.astype(ml_dtypes.bfloat16)  # [n, 128, KT, 65]
    return qkt, vp


def kernel(queries, keys, values, valid_lens):
    queries = np.asarray(queries, np.float32)
    keys = np.asarray(keys, np.float32)
    values = np.asarray(values, np.float32)
    vl = np.asarray(valid_lens).astype(np.int64)

    # Ragged load balancing: sort batches by active k-tile count descending,
    # deal them across cores (slot s <- sorted[s*N_CORES + c]), so each slot
    # runs the max tile count of its group of 8 on every core.
    nact = np.where(vl == 0, KT, -(-vl // 128)).astype(np.int64)
    order = np.argsort(-nact, kind="stable")
    ns = tuple(int(nact[order[s * N_CORES]]) for s in range(BPC))

    qkt, vp = _prep_inputs(queries[order], keys[order], values[order],
                           vl[order])

    nc = get_program(ns)
    in_maps = []
    for c in range(N_CORES):
        idx = [s * N_CORES + c for s in range(BPC)]
        in_maps.append({
            "qkt": np.ascontiguousarray(qkt[idx]),
            "vp": np.ascontiguousarray(vp[idx]),
        })

    res = None
    for attempt in range(3):
        try:
            res = run_bass_kernel_spmd(nc, in_maps, list(range(N_CORES)))
            break
        except Exception:
            # Transient NRT/axon device failures have been observed on the
            # first execution of a freshly compiled NEFF; reset and retry.
            if attempt == 2:
                raise
            import time as _time
            _time.sleep(2.0)
            try:
                import jax
                jax.clear_caches()
            except Exception:
                pass

    out = np.empty((B, L, D), np.float32)
    for c in range(N_CORES):
        o = res.results[c]["o"]  # [BPC, 128, 577]: 8x(64 num + 1 den) regions
        lo = o[:, :, :455].reshape(BPC, 128, 7, 65)
        num = np.concatenate([lo[..., :64], o[:, :, None, 512:576]], axis=2)
        den = np.concatenate([lo[..., 64], o[:, :, None, 576]], axis=2)
        on = (num / den[..., None]).transpose(0, 2, 1, 3).reshape(BPC, L, D)
        for s in range(BPC):
            out[order[s * N_CORES + c]] = on[s]
    return out


# revision 7
# speedup vs baseline: 1.0349x; 1.0034x over previous
"""Masked dot-product attention (B=64, Lq=Lk=1024, d=64, fp32) on 8 TRN2 cores.

Strategy (batch-parallel, 8 batch slots per core), v2:
  - Host folds the 1/sqrt(d) scale into Q and the additive key mask into an
    extra contraction row, so masked scores come out of one matmul:
        S^T[k, q] = sum_d K[k,d] * Q[q,d]/8 + maskadd[k]
    lhsT = ktm k-tile ([65, 128]), rhs = qt ([65, 1024]), fp32r.
  - Per k-tile, exp(S^T) is computed on ONE of two engines (greedy
    load-balanced):
      * ACT: exact exp, PSUM -> SBUF bf16.
      * DVE: Schraudolph-16 — int16(round(s*2^7*log2e + C)) written into a
        bf16 tile via bitcast; the int16 bit pattern IS the bf16 of
        ~0.971*2^(s*log2e) (max rel err ~3.3%).  fp32->int16 conversion
        saturates, so masked scores (-1e6) land at -32768 = bf16 -0.0,
        which contributes exactly nothing to numerator and denominator.
  - O accumulation per q-block j (8 blocks of 128 queries):
        opsum[q, 64j:64j+64] += P-block[k, q].T @ V-tile[k, :64]   (bf16)
        opsum[q, 512+j]      += P-block[k, q].T @ ones[k]          (denoms)
    64-wide bf16 matmuls cost half the columns of the fp32r orientation.
    PSUM keeps ONE open accumulation context per 2KB bank, so the kt loop
    must be INSIDE the j loop (groups sequential per bank); num lives in
    bank 6, den in bank 7 of the opsum tile.
  - Software pipelining: batch b's O-matmuls are emitted after batch b+1's
    first S/exp chunks so the exp engines stay fed during the O burst.
  - opsum [128, 520] is copied PSUM->SBUF (ACT or DVE, balanced) and DMA'd
    out unnormalized; host divides numerators by denominators.
  - Raggedness: k-tiles fully beyond valid_len are dead; batches are sorted
    by active-tile count and dealt across cores; per-slot tile counts baked
    into the program (compiled per distinct count tuple, cached).
  - valid_len==0 batches are host-patched (Q rows zeroed, mask zeroed) so
    scores are all 0 -> uniform attention, matching the reference.
"""

import numpy as np
import ml_dtypes

import concourse.bass as bass
import concourse.mybir as mybir
import concourse.tile as tile
from concourse import bacc
from concourse.bass_utils import run_bass_kernel_spmd

N_CORES = 8
B = 64
L = 1024
D = 64
BPC = B // N_CORES  # batch slots per core
KT = L // 128       # max k-tiles per batch
NEG_INF = -1000000.0

F32 = mybir.dt.float32
F32R = mybir.dt.float32r
BF16 = mybir.dt.bfloat16
I16 = mybir.dt.int16

# Schraudolph-16 constants: int16(round(s*SC1 + SC2)) bit-viewed as bf16
# approximates exp(s) within [0.968, 1.034]x.
SC1 = float(np.float32(184.66504))   # 2**7 * log2(e)
SC2 = float(np.float32(16250.51))    # 127*2**7 + 2**7*log2(0.9707) (minmax)

# Cost-model estimates (ns) used only for static ACT/DVE load balancing.
CHUNK = 512         # exp chunk = half a k-tile strip
ACT_CHUNK = CHUNK * 0.8333 + 185
DVE_CHUNK = CHUNK * 1.0417 + 125
ACT_COPY = 520 * 0.8333 + 185
DVE_COPY = 520 * 1.0417 + 125

_prog_cache = {}


def _build_program(ns):
    """ns: per-slot k-tile counts (tuple of BPC ints in 1..KT)."""
    nc = bacc.Bacc("TRN2", target_bir_lowering=False, debug=False,
                   num_devices=N_CORES)
    # qkt packs [ktm_ktile0 (128) | qt (1024) | ktm_ktile1.. (896)] so a
    # batch's whole Q/K working set arrives in one DMA.
    qkt_d = nc.dram_tensor("qkt", [BPC, D + 1, 2 * L + 128], F32R,
                           kind="ExternalInput")
    vp_d = nc.dram_tensor("vp", [BPC, 128, KT, D + 1], BF16,
                          kind="ExternalInput")
    o_d = nc.dram_tensor("o", [BPC, 128, 2, 260], F32, kind="ExternalOutput")

    # Greedy two-engine balance.  ACT is pre-charged with its activation
    # table load (1.28us, overlaps input DMAs but delays its exp stream).
    busy = {"A": 1280.0, "V": 0.0}

    def pick(cost_a, cost_v):
        if busy["V"] + cost_v <= busy["A"] + cost_a:
            busy["V"] += cost_v
            return "V"
        busy["A"] += cost_a
        return "A"

    with tile.TileContext(nc) as tc:
        with (
            tc.tile_pool(name="qk", bufs=8) as qk_pool,
            tc.tile_pool(name="vpp", bufs=8) as vp_pool,
            tc.tile_pool(name="pt", bufs=28) as pt_pool,
            tc.tile_pool(name="osb", bufs=4) as osb_pool,
            tc.tile_pool(name="wu", bufs=1) as wu_pool,
            tc.tile_pool(name="sp", bufs=4, space="PSUM") as sp_pool,
            tc.tile_pool(name="op", bufs=2, space="PSUM") as op_pool,
        ):
            # PE p-state warmup: a dummy matmul as early as possible starts
            # the 3us ramp clock so real matmuls hit full speed sooner.
            wu = wu_pool.tile([1, 256], BF16)
            nc.vector.memset(wu[:], 0.0)
            wp = sp_pool.tile([128, CHUNK], F32, name="sp")
            nc.tensor.matmul(wp[:, :128], wu[:, :128], wu[:, 128:256],
                             start=True, stop=True)

            def make_o_closures(state):
                """Per-batch O-matmul emission, one j-group per closure.
                kt must be INSIDE j: PSUM keeps one open accumulation
                context per 2KB bank (num groups share bank A of opsum,
                den groups bank B)."""
                b, nkt, pts, vp_s, opsum, tail = state

                def o_group(j):
                    # 65-wide merged num+den matmuls.  j 0-3 accumulate in
                    # PSUM bank A (at 65*j), j 4-7 in bank B; regions never
                    # straddle a bank.  Groups are emitted in bank-
                    # ALTERNATING order so consecutive groups' start/stop
                    # handoffs overlap (one open context per bank).
                    blk, off = j // 4, 65 * (j % 4)

                    def emit():
                        w = 128 * (j % 4)
                        for kt in range(nkt):
                            first, last = kt == 0, kt == nkt - 1
                            pb = pts[2 * kt + j // 4][:, w:w + 128]
                            nc.tensor.matmul(
                                opsum[:, blk, off:off + 65], pb,
                                vp_s[:, kt, :], start=first, stop=last)
                    return emit

                def copy_out():
                    osb = osb_pool.tile([128, 2, 260], F32)
                    if tail:
                        # tail batches: whole copy on one engine each (no
                        # cross-engine rendezvous before the out-DMA); ACT
                        # finishes its exps first and takes the earlier one
                        if b == BPC - 2:
                            nc.vector.tensor_copy(osb[:], opsum[:, :, :260])
                        else:
                            nc.scalar.copy(osb[:], opsum[:, :, :260])
                    else:
                        # DVE's mid-stream idle absorbs copies for free;
                        # keep ACT's (critical) exp stream copy-free
                        nc.vector.tensor_copy(osb[:], opsum[:, :, :260])
                    nc.sync.dma_start(o_d[b], osb[:])

                return [o_group(j) for j in (0, 4, 1, 5, 2, 6, 3, 7)] + \
                    [copy_out]

            gci = 0           # global chunk counter (startup engine forcing)
            prev = None       # completed batch awaiting O emission
            pending = []      # its closures, drained one per chunk
            for b in range(BPC):
                nkt = ns[b]
                tail = b >= BPC - 2
                end = 128 + L + (nkt - 1) * 128
                qkt_s = qk_pool.tile([D + 1, 2 * L + 128], F32R, tag="qkt")
                vp_s = vp_pool.tile([128, KT, D + 1], BF16)
                if b == 0:
                    # split head loads so the first chunks start asap
                    nc.sync.dma_start(qkt_s[:, :640], qkt_d[b][:, :640])
                    nc.sync.dma_start(qkt_s[:, 640:1152],
                                      qkt_d[b][:, 640:1152])
                    if end > 1152:
                        nc.sync.dma_start(qkt_s[:, 1152:end],
                                          qkt_d[b][:, 1152:end])
                else:
                    nc.sync.dma_start(qkt_s[:, :end], qkt_d[b][:, :end])
                nc.sync.dma_start(vp_s[:, :1, :], vp_d[b][:, :1, :])
                if nkt > 1:
                    nc.sync.dma_start(vp_s[:, 1:nkt, :], vp_d[b][:, 1:nkt, :])
                qt_s = qkt_s[:, 128:128 + L]

                def ktm_sl(kt):
                    if kt == 0:
                        return qkt_s[:, :128]
                    o = 128 + L + (kt - 1) * 128
                    return qkt_s[:, o:o + 128]

                opsum = op_pool.tile([128, 2, 512], F32)
                pts = []
                for ci in range(2 * nkt):
                    kt, h = divmod(ci, 2)
                    sp = sp_pool.tile([128, CHUNK], F32)
                    pt = pt_pool.tile([128, CHUNK], BF16)
                    pts.append(pt)
                    nc.tensor.matmul(
                        sp[:], ktm_sl(kt), qt_s[:, h * 512:(h + 1) * 512],
                        start=True, stop=True)
                    # Slots with few active k-tiles hold small-valid_len
                    # batches: few softmax terms, so the Schraudolph ripple
                    # doesn't average out.  Keep those on exact ACT exp.
                    if nkt <= 2:
                        busy["A"] += ACT_CHUNK
                        eng = "A"
                    else:
                        eng = pick(ACT_CHUNK, DVE_CHUNK)
                    gci += 1
                    if eng == "A":
                        nc.scalar.activation(
                            pt[:], sp[:], mybir.ActivationFunctionType.Exp)
                    else:
                        nc.vector.tensor_scalar(
                            pt[:].bitcast(I16), sp[:], SC1, SC2,
                            mybir.AluOpType.mult, mybir.AluOpType.add)
                    if ci >= 3 and pending:
                        pending.pop(0)()
                while pending:
                    pending.pop(0)()
                prev = (b, nkt, pts, vp_s, opsum, tail)
                pending = make_o_closures(prev)

            # final batch: drain its O groups + copy-out
            for cl in pending:
                cl()

    nc.compile()
    return nc


def get_program(ns):
    ns = tuple(ns)
    if ns not in _prog_cache:
        _prog_cache[ns] = _build_program(ns)
    return _prog_cache[ns]


def _prep_inputs(q, k, v, vl):
    """q,k,v: [n, L, D] fp32; vl: [n] int. Returns (qkt, vp) arrays."""
    n = q.shape[0]
    qt = np.empty((n, D + 1, L), np.float32)
    qt[:, :D] = q.transpose(0, 2, 1) * np.float32(1.0 / np.sqrt(D))
    qt[:, D] = 1.0
    ktm = np.empty((n, D + 1, L), np.float32)
    ktm[:, :D] = k.transpose(0, 2, 1)
    iota = np.arange(L)
    ktm[:, D] = np.where(iota[None, :] < vl[:, None], 0.0, NEG_INF)
    # valid_len == 0: reference softmaxes a constant -1e6 row -> uniform.
    # Reproduce by zeroing the logits entirely (Q rows and mask row).
    zmask = vl == 0
    if zmask.any():
        qt[zmask, :D] = 0.0
        ktm[zmask, D] = 0.0
    qkt = np.empty((n, D + 1, 2 * L + 128), np.float32)
    qkt[:, :, :128] = ktm[:, :, :128]
    qkt[:, :, 128:128 + L] = qt
    qkt[:, :, 128 + L:2 * L] = ktm[:, :, 128:]
    qkt[:, :, 2 * L:] = 0.0
    vp = np.empty((n, L, D + 1), np.float32)
    vp[:, :, :D] = v
    vp[:, :, D] = 1.0
    vp = np.ascontiguousarray(
        vp.reshape(n, KT, 128, D + 1).transpose(0, 2, 1, 3))
    vp = vp.astype(ml_dtypes.bfloat16)  # [n, 128, KT, 65]
    return qkt, vp


def kernel(queries, keys, values, valid_lens):
    queries = np.asarray(queries, np.float32)
    keys = np.asarray(keys, np.float32)
    values = np.asarray(values, np.float32)
    vl = np.asarray(valid_lens).astype(np.int64)

    # Ragged load balancing: sort batches by active k-tile count descending,
    # deal them across cores (slot s <- sorted[s*N_CORES + c]), so each slot
    # runs the max tile count of its group of 8 on every core.
    nact = np.where(vl == 0, KT, -(-vl // 128)).astype(np.int64)
    order = np.argsort(-nact, kind="stable")
    ns = tuple(int(nact[order[s * N_CORES]]) for s in range(BPC))

    qkt, vp = _prep_inputs(queries[order], keys[order], values[order],
                           vl[order])

    nc = get_program(ns)
    in_maps = []
    for c in range(N_CORES):
        idx = [s * N_CORES + c for s in range(BPC)]
        in_maps.append({
            "qkt": np.ascontiguousarray(qkt[idx]),
            "vp": np.ascontiguousarray(vp[idx]),
        })

    res = None
    for attempt in range(3):
        try:
            res = run_bass_kernel_spmd(nc, in_maps, list(range(N_CORES)))
            break
        except Exception:
            # Transient NRT/axon device failures have been observed on the
            # first execution of a freshly compiled NEFF; reset and retry.
            if attempt == 2:
                raise
            import time as _time
            _time.sleep(2.0)
            try:
                import jax
                jax.clear_caches()
            except Exception:
                pass

    out = np.empty((B, L, D), np.float32)
    for c in range(N_CORES):
        o = res.results[c]["o"]  # [BPC, 128, 2, 260]: per bank 4x(64+1)
        r = o.reshape(BPC, 128, 2, 4, 65)
        num = r[..., :64].transpose(0, 2, 3, 1, 4).reshape(BPC, 8, 128, D)
        den = r[..., 64].transpose(0, 2, 3, 1).reshape(BPC, 8, 128)
        on = (num / den[..., None]).reshape(BPC, L, D)
        for s in range(BPC):
            out[order[s * N_CORES + c]] = on[s]
    return out


# revision 8
# speedup vs baseline: 1.0399x; 1.0049x over previous
"""Masked dot-product attention (B=64, Lq=Lk=1024, d=64, fp32) on 8 TRN2 cores.

Strategy (batch-parallel, 8 batch slots per core), v2:
  - Host folds the 1/sqrt(d) scale into Q and the additive key mask into an
    extra contraction row, so masked scores come out of one matmul:
        S^T[k, q] = sum_d K[k,d] * Q[q,d]/8 + maskadd[k]
    lhsT = ktm k-tile ([65, 128]), rhs = qt ([65, 1024]), fp32r.
  - Per k-tile, exp(S^T) is computed on ONE of two engines (greedy
    load-balanced):
      * ACT: exact exp, PSUM -> SBUF bf16.
      * DVE: Schraudolph-16 — int16(round(s*2^7*log2e + C)) written into a
        bf16 tile via bitcast; the int16 bit pattern IS the bf16 of
        ~0.971*2^(s*log2e) (max rel err ~3.3%).  fp32->int16 conversion
        saturates, so masked scores (-1e6) land at -32768 = bf16 -0.0,
        which contributes exactly nothing to numerator and denominator.
  - O accumulation per q-block j (8 blocks of 128 queries):
        opsum[q, 64j:64j+64] += P-block[k, q].T @ V-tile[k, :64]   (bf16)
        opsum[q, 512+j]      += P-block[k, q].T @ ones[k]          (denoms)
    64-wide bf16 matmuls cost half the columns of the fp32r orientation.
    PSUM keeps ONE open accumulation context per 2KB bank, so the kt loop
    must be INSIDE the j loop (groups sequential per bank); num lives in
    bank 6, den in bank 7 of the opsum tile.
  - Software pipelining: batch b's O-matmuls are emitted after batch b+1's
    first S/exp chunks so the exp engines stay fed during the O burst.
  - opsum [128, 520] is copied PSUM->SBUF (ACT or DVE, balanced) and DMA'd
    out unnormalized; host divides numerators by denominators.
  - Raggedness: k-tiles fully beyond valid_len are dead; batches are sorted
    by active-tile count and dealt across cores; per-slot tile counts baked
    into the program (compiled per distinct count tuple, cached).
  - valid_len==0 batches are host-patched (Q rows zeroed, mask zeroed) so
    scores are all 0 -> uniform attention, matching the reference.
"""

import numpy as np
import ml_dtypes

import concourse.bass as bass
import concourse.mybir as mybir
import concourse.tile as tile
from concourse import bacc
from concourse.bass_utils import run_bass_kernel_spmd

N_CORES = 8
B = 64
L = 1024
D = 64
BPC = B // N_CORES  # batch slots per core
KT = L // 128       # max k-tiles per batch
NEG_INF = -1000000.0

F32 = mybir.dt.float32
F32R = mybir.dt.float32r
BF16 = mybir.dt.bfloat16
I16 = mybir.dt.int16

# Schraudolph-16 constants: int16(round(s*SC1 + SC2)) bit-viewed as bf16
# approximates exp(s) within [0.968, 1.034]x.
SC1 = float(np.float32(184.66504))   # 2**7 * log2(e)
SC2 = float(np.float32(16250.51))    # 127*2**7 + 2**7*log2(0.9707) (minmax)

# Cost-model estimates (ns) used only for static ACT/DVE load balancing.
CHUNK = 512         # exp chunk = half a k-tile strip
ACT_CHUNK = CHUNK * 0.8333 + 185
DVE_CHUNK = CHUNK * 1.0417 + 125
ACT_COPY = 520 * 0.8333 + 185
DVE_COPY = 520 * 1.0417 + 125

_prog_cache = {}


def _build_program(ns):
    """ns: per-slot k-tile counts (tuple of BPC ints in 1..KT)."""
    nc = bacc.Bacc("TRN2", target_bir_lowering=False, debug=False,
                   num_devices=N_CORES)
    # qkt packs [ktm_ktile0 (128) | qt (1024) | ktm_ktile1.. (896)] so a
    # batch's whole Q/K working set arrives in one DMA.
    qkt_d = nc.dram_tensor("qkt", [BPC, D + 1, 2 * L + 128], F32R,
                           kind="ExternalInput")
    vp_d = nc.dram_tensor("vp", [BPC, 128, KT, D + 1], BF16,
                          kind="ExternalInput")
    o_d = nc.dram_tensor("o", [BPC, 128, 2, 260], F32, kind="ExternalOutput")

    # Greedy two-engine balance.  ACT is pre-charged with its activation
    # table load (1.28us, overlaps input DMAs but delays its exp stream).
    busy = {"A": 1280.0, "V": 0.0}

    def pick(cost_a, cost_v):
        if busy["V"] + cost_v <= busy["A"] + cost_a:
            busy["V"] += cost_v
            return "V"
        busy["A"] += cost_a
        return "A"

    with tile.TileContext(nc) as tc:
        with (
            tc.tile_pool(name="qk", bufs=8) as qk_pool,
            tc.tile_pool(name="vpp", bufs=8) as vp_pool,
            tc.tile_pool(name="pt", bufs=28) as pt_pool,
            tc.tile_pool(name="osb", bufs=4) as osb_pool,
            tc.tile_pool(name="wu", bufs=1) as wu_pool,
            tc.tile_pool(name="sp", bufs=4, space="PSUM") as sp_pool,
            tc.tile_pool(name="op", bufs=2, space="PSUM") as op_pool,
        ):
            # PE p-state warmup: a dummy matmul as early as possible starts
            # the 3us ramp clock so real matmuls hit full speed sooner.
            wu = wu_pool.tile([1, 256], BF16)
            nc.vector.memset(wu[:], 0.0)
            wp = sp_pool.tile([128, CHUNK], F32, name="sp")
            nc.tensor.matmul(wp[:, :128], wu[:, :128], wu[:, 128:256],
                             start=True, stop=True)

            def make_o_closures(state):
                """Per-batch O-matmul emission, one j-group per closure.
                kt must be INSIDE j: PSUM keeps one open accumulation
                context per 2KB bank (num groups share bank A of opsum,
                den groups bank B)."""
                b, nkt, pts, vp_s, opsum, tail = state

                def o_group(j):
                    # 65-wide merged num+den matmuls.  j 0-3 accumulate in
                    # PSUM bank A (at 65*j), j 4-7 in bank B; regions never
                    # straddle a bank.  Groups are emitted in bank-
                    # ALTERNATING order so consecutive groups' start/stop
                    # handoffs overlap (one open context per bank).
                    blk, off = j // 4, 65 * (j % 4)

                    def emit():
                        w = 128 * (j % 4)
                        for kt in range(nkt):
                            first, last = kt == 0, kt == nkt - 1
                            pb = pts[2 * kt + j // 4][:, w:w + 128]
                            nc.tensor.matmul(
                                opsum[:, blk, off:off + 65], pb,
                                vp_s[:, kt, :], start=first, stop=last)
                    return emit

                def copy_out():
                    osb = osb_pool.tile([128, 2, 260], F32)
                    if tail:
                        # tail batches: whole copy on one engine each (no
                        # cross-engine rendezvous before the out-DMA); ACT
                        # finishes its exps first and takes the earlier one
                        if b == BPC - 2:
                            nc.vector.tensor_copy(osb[:], opsum[:, :, :260])
                        else:
                            nc.scalar.copy(osb[:], opsum[:, :, :260])
                    else:
                        # DVE's mid-stream idle absorbs copies for free;
                        # keep ACT's (critical) exp stream copy-free
                        nc.vector.tensor_copy(osb[:], opsum[:, :, :260])
                    nc.sync.dma_start(o_d[b], osb[:])

                return [o_group(j) for j in (0, 4, 1, 5, 2, 6, 3, 7)] + \
                    [copy_out]

            gci = 0           # global chunk counter (startup engine forcing)
            prev = None       # completed batch awaiting O emission
            pending = []      # its closures, drained one per chunk
            for b in range(BPC):
                nkt = ns[b]
                tail = b >= BPC - 2
                end = 128 + L + (nkt - 1) * 128
                qkt_s = qk_pool.tile([D + 1, 2 * L + 128], F32R, tag="qkt")
                vp_s = vp_pool.tile([128, KT, D + 1], BF16)
                if b == 0:
                    # split head loads so the first chunks start asap
                    nc.sync.dma_start(qkt_s[:, :640], qkt_d[b][:, :640])
                    nc.sync.dma_start(qkt_s[:, 640:1152],
                                      qkt_d[b][:, 640:1152])
                    if end > 1152:
                        nc.sync.dma_start(qkt_s[:, 1152:end],
                                          qkt_d[b][:, 1152:end])
                else:
                    nc.sync.dma_start(qkt_s[:, :end], qkt_d[b][:, :end])
                nc.sync.dma_start(vp_s[:, :1, :], vp_d[b][:, :1, :])
                if nkt > 1:
                    nc.sync.dma_start(vp_s[:, 1:nkt, :], vp_d[b][:, 1:nkt, :])
                qt_s = qkt_s[:, 128:128 + L]

                def ktm_sl(kt):
                    if kt == 0:
                        return qkt_s[:, :128]
                    o = 128 + L + (kt - 1) * 128
                    return qkt_s[:, o:o + 128]

                opsum = op_pool.tile([128, 2, 512], F32)
                pts = []
                for ci in range(2 * nkt):
                    kt, h = divmod(ci, 2)
                    sp = sp_pool.tile([128, CHUNK], F32)
                    pt = pt_pool.tile([128, CHUNK], BF16)
                    pts.append(pt)
                    nc.tensor.matmul(
                        sp[:], ktm_sl(kt), qt_s[:, h * 512:(h + 1) * 512],
                        start=True, stop=True)
                    # Slots with few active k-tiles hold small-valid_len
                    # batches: few softmax terms, so the Schraudolph ripple
                    # doesn't average out.  Keep those on exact ACT exp.
                    if nkt <= 1:
                        busy["A"] += ACT_CHUNK
                        eng = "A"
                    else:
                        eng = pick(ACT_CHUNK, DVE_CHUNK)
                    gci += 1
                    if eng == "A":
                        nc.scalar.activation(
                            pt[:], sp[:], mybir.ActivationFunctionType.Exp)
                    else:
                        nc.vector.tensor_scalar(
                            pt[:].bitcast(I16), sp[:], SC1, SC2,
                            mybir.AluOpType.mult, mybir.AluOpType.add)
                    if ci >= 3 and pending:
                        pending.pop(0)()
                while pending:
                    pending.pop(0)()
                prev = (b, nkt, pts, vp_s, opsum, tail)
                pending = make_o_closures(prev)

            # final batch: drain its O groups + copy-out
            for cl in pending:
                cl()

    nc.compile()
    return nc


def get_program(ns):
    ns = tuple(ns)
    if ns not in _prog_cache:
        _prog_cache[ns] = _build_program(ns)
    return _prog_cache[ns]


def _prep_inputs(q, k, v, vl):
    """q,k,v: [n, L, D] fp32; vl: [n] int. Returns (qkt, vp) arrays."""
    n = q.shape[0]
    qt = np.empty((n, D + 1, L), np.float32)
    qt[:, :D] = q.transpose(0, 2, 1) * np.float32(1.0 / np.sqrt(D))
    qt[:, D] = 1.0
    ktm = np.empty((n, D + 1, L), np.float32)
    ktm[:, :D] = k.transpose(0, 2, 1)
    iota = np.arange(L)
    ktm[:, D] = np.where(iota[None, :] < vl[:, None], 0.0, NEG_INF)
    # valid_len == 0: reference softmaxes a constant -1e6 row -> uniform.
    # Reproduce by zeroing the logits entirely (Q rows and mask row).
    zmask = vl == 0
    if zmask.any():
        qt[zmask, :D] = 0.0
        ktm[zmask, D] = 0.0
    qkt = np.empty((n, D + 1, 2 * L + 128), np.float32)
    qkt[:, :, :128] = ktm[:, :, :128]
    qkt[:, :, 128:128 + L] = qt
    qkt[:, :, 128 + L:2 * L] = ktm[:, :, 128:]
    qkt[:, :, 2 * L:] = 0.0
    vp = np.empty((n, L, D + 1), np.float32)
    vp[:, :, :D] = v
    vp[:, :, D] = 1.0
    vp = np.ascontiguousarray(
        vp.reshape(n, KT, 128, D + 1).transpose(0, 2, 1, 3))
    vp = vp.astype(ml_dtypes.bfloat16)  # [n, 128, KT, 65]
    return qkt, vp


def kernel(queries, keys, values, valid_lens):
    queries = np.asarray(queries, np.float32)
    keys = np.asarray(keys, np.float32)
    values = np.asarray(values, np.float32)
    vl = np.asarray(valid_lens).astype(np.int64)

    # Ragged load balancing: sort batches by active k-tile count descending,
    # deal them across cores (slot s <- sorted[s*N_CORES + c]), so each slot
    # runs the max tile count of its group of 8 on every core.
    nact = np.where(vl == 0, KT, -(-vl // 128)).astype(np.int64)
    order = np.argsort(-nact, kind="stable")
    ns = tuple(int(nact[order[s * N_CORES]]) for s in range(BPC))

    qkt, vp = _prep_inputs(queries[order], keys[order], values[order],
                           vl[order])

    nc = get_program(ns)
    in_maps = []
    for c in range(N_CORES):
        idx = [s * N_CORES + c for s in range(BPC)]
        in_maps.append({
            "qkt": np.ascontiguousarray(qkt[idx]),
            "vp": np.ascontiguousarray(vp[idx]),
        })

    res = None
    for attempt in range(3):
        try:
            res = run_bass_kernel_spmd(nc, in_maps, list(range(N_CORES)))
            break
        except Exception:
            # Transient NRT/axon device failures have been observed on the
            # first execution of a freshly compiled NEFF; reset and retry.
            if attempt == 2:
                raise
            import time as _time
            _time.sleep(2.0)
            try:
                import jax
                jax.clear_caches()
            except Exception:
                pass

    out = np.empty((B, L, D), np.float32)
    for c in range(N_CORES):
        o = res.results[c]["o"]  # [BPC, 128, 2, 260]: per bank 4x(64+1)
        r = o.reshape(BPC, 128, 2, 4, 65)
        num = r[..., :64].transpose(0, 2, 3, 1, 4).reshape(BPC, 8, 128, D)
        den = r[..., 64].transpose(0, 2, 3, 1).reshape(BPC, 8, 128)
        on = (num / den[..., None]).reshape(BPC, L, D)
        for s in range(BPC):
            out[order[s * N_CORES + c]] = on[s]
    return out
